# revision 26
# baseline (speedup 1.0000x reference)
"""Distributed attention kernel for 8 TRN2 NeuronCores (~260-276us across
runs, mean ~269us, from a 349.9us baseline; rel err 1.347e-2 vs the 2e-2
gate).

Problem: x[2,2048,1024] -> qkv proj -> 16-head attention (softmax then /scale
quirk) -> out proj + bias. Core c owns heads {2c, 2c+1} for both batches.

Design, driven by perfetto traces of the baseline:

1. ALL-BF16 compute (weights, activations, scores; f32 PSUM accumulation).
   The attention region is ScalarE(exp)-bound (~1.3us per [128,1024] exp vs
   ~0.95us of PE per k-chunk), so PE dtype speed mostly buys schedule slack;
   bf16 also halves DMA bytes and SBUF. Emulated + measured end-to-end error
   1.346e-2. Do NOT mix bf16 and f32r matmuls (nondeterministic weight
   corruption on HW). ROW_TILE_QK=False: 64-row tile_position matmul pairs
   were measured SERIAL and HAM-cold (~400ns each) - zero-padded full-array
   per-head qk (the q tiles carry the other head's rows as zeros) is faster.
2. Merged-head attention loop: per (batch, 512-qpos block, k-chunk) both
   heads' logits land in one [128, 2x512] PSUM tile covered by a SINGLE exp
   whose output feeds score@v as the moving operand (logits computed
   transposed, so no on-chip transpose of the attention matrix); score@v is
   software-pipelined one chunk behind exp, and an appended ones-column in
   the v stationary yields softmax denominators for free. PSUM: 2x2-bank lt
   (double-buffered) + 2x1-bank outT + 2 chain banks = exactly 8.
3. DMA is DESCRIPTOR-bound, not byte-bound (~144 descriptors per
   128-partition transfer regardless of size): w_qkv/w_out are host-packed
   into single [128, N] transfers, x loads as one tile per (batch, k-chunk)
   in 3 column-waves matched to the nkk-major QKV chain order.
4. AllToAll carries only useful data: per (batch, qpos-half) a
   [8 slots, 128 rows, 128 tokens] exchange fired mid-attention (qh1) and
   at the end (qh3); every core projects 2x128 tokens per batch against the
   full w_out (no discarded projections, w_out needs no permutation: sender
   s's rows are heads 2s,2s+1 = w_out rows 128s..128s+128). Collectives
   have a ~10-15us latency floor and ~30us cold start: a throwaway warmup
   A2A runs under the QKV phase and per-(b,half) DRAM tiles keep the Tile
   dependency tracker from serializing readers on unrelated collectives.
   (Splitting the final exchange into two quarter-calls was tried and
   REVERTED: consecutive collectives serialize on the CC engine, so two
   small tail calls finish LATER than one 256KB call.)
5. Scheduling: both QKV phases run serially up front (the static Tile
   scheduler reorders "filler" emission anyway; attention then runs at the
   exp floor), projection work for batch b weaves into batch b+1's
   ACT-bound attention via rate-limited filler generators placed only at
   qh-blocks where the producing collective is guaranteed complete (a
   filler matmul queued behind an in-flight collective head-of-line blocks
   the whole PE stream). Normalization evacuates PSUM on the Vector engine
   (keeping ScalarE exp-only).
6. DVE's stock RECIPROCAL is column-serial (~3.9us per 512 columns
   regardless of partition count); 16 of them congested the Vector engine
   enough to delay QKV-chain PSUM evacuations, head-of-line stalling the PE
   behind ps_a rotation and amplifying cross-core collective skew (runs
   varied 278-321us). reciprocal_approx_fast (51-ULP custom-DVE op at ALU
   rate) cut that to ~0.5us each - mean dropped ~30us and the run-to-run
   spread tightened to ~±8us. Note custom-DVE ops (unlike stock ops) cannot
   read from a shifted base partition: the denominator row is stock-copied
   from partition 64 to a partition-0 tile first (doing otherwise returns
   garbage, not an error).
"""

import numpy as np

S = 2048          # sequence length
D = 1024          # model dim
NH = 16           # total heads
DH = 64           # head dim
HPC = 2           # heads per core
NCORES = 8
KC = 8            # k-chunks of D (128 each)
NK = S // 128     # kpos chunks per batch (16)
NQH = 4           # qpos blocks of 512 per batch
SCALE_INV = 8.0   # 1 / (DH ** -0.5)

USE_BF16 = True
ROW_TILE_QK = False

_CACHE = {}


def _ensure_paths():
    import sys
    for p in ("/opt/trn_rl_repo", "/root/.axon_site"):
        if p not in sys.path:
            sys.path.insert(0, p)


def _build_nc():
    _ensure_paths()
    from contextlib import ExitStack
    import concourse.bass as bass
    import concourse.mybir as mybir
    import concourse.tile as tile
    from concourse import bacc
    from concourse.masks import make_identity

    f32 = mybir.dt.float32
    DT = mybir.dt.bfloat16 if USE_BF16 else mybir.dt.float32r
    DTT = mybir.dt.bfloat16 if USE_BF16 else f32  # transpose-path dtype
    EXP = mybir.ActivationFunctionType.Exp

    nc = bacc.Bacc(None)
    xT_ext = nc.declare_dram_parameter("xT", [2, KC, 128, S], DT, isOutput=False)
    wq_ext = nc.declare_dram_parameter("w_qkv", [128, KC * 3 * 128], DT, isOutput=False)
    wo_ext = nc.declare_dram_parameter("w_out", [128, KC * D], DT, isOutput=False)
    bout_ext = nc.declare_dram_parameter("b_out", [D], f32, isOutput=False)
    out_ext = nc.declare_dram_parameter("out", [2, 2, 128, D], f32, isOutput=True)

    with tile.TileContext(nc) as tc, ExitStack() as ctx:
        ctx.enter_context(
            nc.allow_low_precision(reason="f32r/bf16 storage throughout")
        )
        const = ctx.enter_context(tc.tile_pool(name="const", bufs=1))
        qk_pool = ctx.enter_context(tc.tile_pool(name="qk", bufs=8))
        vt_pool = ctx.enter_context(tc.tile_pool(name="vt", bufs=1))
        vo_pool = ctx.enter_context(tc.tile_pool(name="vo", bufs=32))
        st_pool = ctx.enter_context(tc.tile_pool(name="st", bufs=2))
        stage_pool = ctx.enter_context(tc.tile_pool(name="stg", bufs=2))
        ot_pool = ctx.enter_context(tc.tile_pool(name="ot", bufs=4))
        rcp_pool = ctx.enter_context(tc.tile_pool(name="rcp", bufs=4))
        bc_pool = ctx.enter_context(tc.tile_pool(name="bc", bufs=4))

        ps_lt = ctx.enter_context(tc.tile_pool(name="psLT", bufs=2, space="PSUM"))
        ps_ot = ctx.enter_context(tc.tile_pool(name="psOT", bufs=2, space="PSUM"))
        ps_a = ctx.enter_context(tc.tile_pool(name="psA", bufs=2, space="PSUM"))
        dram = ctx.enter_context(tc.tile_pool(name="dram", bufs=1, space="DRAM"))

        a2a_in = {}
        a2a_out = {}
        for q in ("q2", "q3"):
            a2a_in[1, q] = dram.tile(
                [NCORES, 128, 64], DT, tag=f"a2ai{q}", name=f"a2a_in{q}"
            )
            a2a_out[1, q] = dram.tile(
                [NCORES, 128, 64], DT, tag=f"a2ao{q}", name=f"a2a_out{q}"
            )
        for bb in range(2):
            for hh in range(2):
                a2a_in[bb, hh] = dram.tile(
                    [NCORES, 128, 128], DT, tag=f"a2ai{bb}{hh}", name=f"a2a_in{bb}{hh}"
                )
                a2a_out[bb, hh] = dram.tile(
                    [NCORES, 128, 128], DT, tag=f"a2ao{bb}{hh}", name=f"a2a_out{bb}{hh}"
                )
        cc_warm_in = dram.tile([NCORES, 128], DT, tag="ccw_i", name="ccw_i")
        cc_warm_out = dram.tile([NCORES, 128], DT, tag="ccw_o", name="ccw_o")

        # ---- constants ----
        ident = const.tile([128, 128], DTT, tag="ident", name="ident")
        make_identity(nc, ident)
        ones2 = const.tile([128, HPC, 1], DTT, tag="ones2", name="ones2")
        nc.vector.memset(ones2, 1.0)
        zeros2 = const.tile([128, HPC, 128 - DH - 1], DTT, tag="zeros2", name="zeros2")
        nc.vector.memset(zeros2, 0.0)
        zpad = const.tile([DH, 512], DT, tag="zpad", name="zpad")
        zscr = const.tile([DH, 512], f32, tag="zscr", name="zscr")
        nc.vector.memset(zscr, 0.0)
        nc.vector.tensor_copy(zpad, zscr)
        # pre-warm the exp table set so the ~2.7us ACT_TABLE_LOAD overlaps the
        # DMA-gated QKV phase instead of the first attention chunk
        warm = const.tile([1, 2], f32, tag="warm", name="warm")
        nc.vector.memset(warm, 0.0)
        nc.scalar.activation(warm, warm, EXP)

        def load_bias(pool):
            bias_sb = pool.tile([128, D], f32, tag="bias", name="bias_sb")
            bias_ap = bout_ext.ap()
            bias_bcast = bass.AP(
                tensor=bias_ap.tensor,
                offset=bias_ap.offset,
                ap=[[0, 128]] + [list(p) for p in bias_ap.ap],
            )
            nc.sync.dma_start(out=bias_sb, in_=bias_bcast)
            return bias_sb

        qT = {}
        kT = {}
        vo = {}
        stage = {}

        def drain(it, n=None):
            if it is None:
                return
            if n is None:
                for _ in it:
                    pass
            else:
                for _ in range(n):
                    if next(it, StopIteration) is StopIteration:
                        break

        def chain(*gens):
            for g in gens:
                if g is not None:
                    yield from g

        def load_x(b, xt_pool):
            # one [128,2048] tile per k-chunk, filled in nkk-major waves so
            # the n-th accumulation chain only waits for the n-th wave
            xts = []
            for k in range(KC):
                t = xt_pool.tile([128, S], DT, tag="xt", name=f"xt{b}_{k}")
                xts.append(t)
            for k in range(KC):
                nc.sync.dma_start(out=xts[k][:, 0:512], in_=xT_ext[b, k][:, 0:512])
            for k in range(KC):
                nc.sync.dma_start(out=xts[k][:, 512:1024], in_=xT_ext[b, k][:, 512:1024])
            for k in range(KC):
                nc.sync.dma_start(out=xts[k][:, 1024:S], in_=xT_ext[b, k][:, 1024:S])
            return xts

        def qkv_chains(b, wq_sb, xts, secs):
            # one generator step per matmul / copy; sections: 0=q, 1=k, 2=v
            for sec, nkks in secs:
                if sec == 0:
                    dst = qT[b]
                elif sec == 1:
                    dst = kT[b]
                else:
                    if b not in vt_cur:
                        vt_cur[b] = vt_pool.tile([128, S], DTT, tag="vt", name=f"vT{b}")
                    dst = vt_cur[b]
                for nkk in nkks:
                    ps = ps_a.tile([128, 512], f32, tag="psA", name=f"qkv{b}_{sec}_{nkk}")
                    for k in range(KC):
                        nc.tensor.matmul(
                            ps,
                            lhsT=wq_sb[:, k * 384 + sec * 128:k * 384 + sec * 128 + 128],
                            rhs=xts[k][:, nkk * 512:(nkk + 1) * 512],
                            start=(k == 0),
                            stop=(k == KC - 1),
                        )
                        yield
                    if sec == 0 and not ROW_TILE_QK:
                        c0 = nkk * 512
                        for h in range(HPC):
                            nc.vector.tensor_copy(
                                qT[b][h][h * DH:(h + 1) * DH, c0:c0 + 512],
                                ps[h * DH:(h + 1) * DH, :],
                            )
                    else:
                        nc.vector.tensor_copy(dst[:, nkk * 512:(nkk + 1) * 512], ps)
                    yield
                if sec == 2 and nkks[-1] == 3:
                    vT = vt_cur[b]
                    for sc in range(NK):
                        vps = ps_a.tile([128, 128], DTT, tag="psA", name=f"vps{b}_{sc}")
                        nc.tensor.transpose(vps, vT[:, sc * 128:(sc + 1) * 128], ident)
                        vt = vo_pool.tile([128, HPC, 128], DT, tag="vo", name=f"vo{b}_{sc}")
                        nc.vector.tensor_copy(
                            vt[:, :, 0:DH], vps.rearrange("p (h d) -> p h d", h=HPC)
                        )
                        nc.vector.tensor_copy(vt[:, :, DH:DH + 1], ones2)
                        nc.vector.tensor_copy(vt[:, :, DH + 1:], zeros2)
                        vo[b][sc] = vt
                        yield

        vt_cur = {}

        def qkv_start(b, wq_sb, xts, split=False):
            if ROW_TILE_QK:
                qT[b] = qk_pool.tile([128, S], DT, tag="qk", name=f"qT{b}")
            else:
                qT[b] = [
                    qk_pool.tile([128, S], DT, tag="qk", name=f"qT{b}_{h}")
                    for h in range(HPC)
                ]
                for h in range(HPC):
                    r0 = DH * (1 - h)
                    for c in range(4):
                        nc.vector.tensor_copy(
                            qT[b][h][r0:r0 + DH, c * 512:(c + 1) * 512], zpad
                        )
            kT[b] = qk_pool.tile([128, S], DT, tag="qk", name=f"kT{b}")
            vo[b] = [None] * NK
            if split:
                # qh0 only needs q cols 0:512 - defer the q1-3 chains into
                # the attention filler stream so attention starts ~3 chains
                # earlier (kT/v/transposes must still fully precede it)
                eager = [(1, [0]), (2, [0]), (0, [0])] + [
                    (sec, [nkk]) for nkk in (1, 2, 3) for sec in (1, 2)
                ]
                return (
                    qkv_chains(b, wq_sb, xts, eager),
                    qkv_chains(b, wq_sb, xts, [(0, [1]), (0, [2]), (0, [3])]),
                )
            order = [(sec, [nkk]) for nkk in range(4) for sec in (1, 2, 0)]
            return qkv_chains(b, wq_sb, xts, order)

        def attention(b, fillers, rates):
            # fillers: {qh: generator} appended to the live filler at that block
            live = []
            stage[b] = stage_pool.tile([128, S], DT, tag="stg", name=f"stg{b}")

            def filler_step(n):
                for _ in range(n):
                    while live:
                        if next(live[0], StopIteration) is StopIteration:
                            live.pop(0)
                        else:
                            break
                    if not live:
                        return

            for qh in range(NQH):
                q0 = qh * 512
                rate = rates[qh]
                if fillers.get(qh) is not None:
                    live.append(fillers[qh])
                outT = [
                    ps_ot.tile([128, 512], f32, tag="psOT", name=f"oT{b}_{qh}_{h}")
                    for h in range(HPC)
                ]

                def sv(k, st):
                    # 65-col stationary (v + ones, no zero pad): halves the
                    # LDWEIGHTS exposure; HAM stays warm off the full-array
                    # qk matmuls every chunk
                    for h in range(HPC):
                        nc.tensor.matmul(
                            outT[h][0:DH + 1, :],
                            lhsT=vo[b][k][:, h, 0:DH + 1],
                            rhs=st[:, h * 512:(h + 1) * 512],
                            start=(k == 0),
                            stop=(k == NK - 1),
                        )

                pending = None
                for k in range(NK):
                    lt = ps_lt.tile([128, 1024], f32, tag="psLT", name=f"lt{b}_{qh}_{k}")
                    for h in range(HPC):
                        if ROW_TILE_QK:
                            nc.tensor.matmul(
                                lt[:, h * 512:(h + 1) * 512],
                                lhsT=kT[b][h * DH:(h + 1) * DH, k * 128:(k + 1) * 128],
                                rhs=qT[b][h * DH:(h + 1) * DH, q0:q0 + 512],
                                start=True,
                                stop=True,
                                tile_position=(h * DH, 0),
                            )
                        else:
                            nc.tensor.matmul(
                                lt[:, h * 512:(h + 1) * 512],
                                lhsT=kT[b][:, k * 128:(k + 1) * 128],
                                rhs=qT[b][h][:, q0:q0 + 512],
                                start=True,
                                stop=True,
                            )
                    st = st_pool.tile([128, 1024], DT, tag="st", name=f"st{b}_{qh}_{k}")
                    nc.scalar.activation(st, lt, EXP)
                    if pending is not None:
                        sv(*pending)
                    pending = (k, st)
                    filler_step(rate)
                sv(*pending)
                # normalize both heads into stage (evac on DVE, not ScalarE:
                # ScalarE is the exp-bottleneck engine)
                for h in range(HPC):
                    ot = ot_pool.tile([DH + 1, 512], f32, tag="ot", name=f"ot{b}_{qh}_{h}")
                    bc = bc_pool.tile([DH, 512], f32, tag="bc", name=f"bc{b}_{qh}_{h}")
                    nc.vector.tensor_copy(ot, outT[h][0:DH + 1, :])
                    # DVE's stock RECIPROCAL is column-serial (~3.9us for 512
                    # cols regardless of partitions); the 51-ULP custom-DVE
                    # approx runs at ALU rate and is far beyond bf16 accuracy
                    dn = rcp_pool.tile([1, 512], f32, tag="rcp", name=f"dn{b}_{qh}_{h}")
                    nc.vector.tensor_copy(dn, ot[DH:DH + 1, :])
                    recip = rcp_pool.tile([1, 512], f32, tag="rcp", name=f"rcp{b}_{qh}_{h}")
                    nc.vector.reciprocal_approx_fast(out=recip, in_=dn)
                    nc.gpsimd.partition_broadcast(bc, recip)
                    nc.vector.scalar_tensor_tensor(
                        out=stage[b][h * DH:(h + 1) * DH, q0:q0 + 512],
                        in0=ot[0:DH, :],
                        scalar=SCALE_INV,
                        in1=bc,
                        op0=mybir.AluOpType.mult,
                        op1=mybir.AluOpType.mult,
                    )
                key = None
                if qh == 1:
                    key, lo, w = (b, 0), 0, 128
                elif qh == 3:
                    key, lo, w = (b, 1), 1024, 128
                if key is not None:
                    for s in range(NCORES):
                        nc.sync.dma_start(
                            out=a2a_in[key][s],
                            in_=stage[b][:, lo + s * w:lo + (s + 1) * w],
                        )
                    nc.gpsimd.collective_compute(
                        "AllToAll",
                        mybir.AluOpType.bypass,
                        replica_groups=[list(range(NCORES))],
                        ins=[a2a_in[key].opt()],
                        outs=[a2a_out[key].opt()],
                    )

        def proj_gen(b, half, wo_sb, bias_sb, g_pool, y_pool):
            g_sb = []
            for s in range(NCORES):
                t = g_pool.tile([128, 128], DT, tag="g", name=f"g{b}_{half}_{s}")
                nc.sync.dma_start(out=t, in_=a2a_out[b, half][s])
                g_sb.append(t)

            def gen():
                y_sb = y_pool.tile([128, D], f32, tag="y", name=f"y{b}_{half}")
                for nk in range(2):
                    yps = ps_a.tile([128, 512], f32, tag="psA", name=f"yps{b}_{half}_{nk}")
                    for s in range(NCORES):
                        nc.tensor.matmul(
                            yps,
                            lhsT=g_sb[s],
                            rhs=wo_sb[:, s * D + nk * 512:s * D + (nk + 1) * 512],
                            start=(s == 0),
                            stop=(s == NCORES - 1),
                        )
                        yield
                    nc.vector.tensor_add(
                        y_sb[:, nk * 512:(nk + 1) * 512],
                        yps,
                        bias_sb[:, nk * 512:(nk + 1) * 512],
                    )
                    yield
                nc.sync.dma_start(out=out_ext[b, half], in_=y_sb)
                yield

            return gen()

        with tc.tile_pool(name="xt", bufs=16) as xt_pool, \
                tc.tile_pool(name="wq", bufs=1) as wq_pool:
            wq_sb = wq_pool.tile([128, KC * 3 * 128], DT, tag="wq", name="wq")
            nc.sync.dma_start(out=wq_sb, in_=wq_ext.ap())
            # tiny throwaway AllToAll: the first collective pays ~30us of
            # cold-start; absorb it under the DMA-gated QKV phase
            nc.gpsimd.dma_start(out=cc_warm_in[0:1], in_=ident[0:1, :])
            nc.gpsimd.collective_compute(
                "AllToAll",
                mybir.AluOpType.bypass,
                replica_groups=[list(range(NCORES))],
                ins=[cc_warm_in.opt()],
                outs=[cc_warm_out.opt()],
            )
            xts0 = load_x(0, xt_pool)
            eager0, tail0 = qkv_start(0, wq_sb, xts0, split=True)
            drain(eager0)
            xts1 = load_x(1, xt_pool)
            g1 = qkv_start(1, wq_sb, xts1)
            # NB: the chain must outlive attention() and be drained itself -
            # if the anonymous generator is GC'd, close() propagates through
            # the suspended `yield from` and kills g1 silently
            f0 = chain(tail0, g1)
            attention(0, {0: f0}, rates=(2, 2, 2, 2))
            drain(f0)
        # xt/wq freed: projection pools fit alongside the attention pools
        wo_pool = ctx.enter_context(tc.tile_pool(name="wo", bufs=1))
        g_pool = ctx.enter_context(tc.tile_pool(name="g", bufs=4 * NCORES))
        y_pool = ctx.enter_context(tc.tile_pool(name="y", bufs=2))
        bias_sb = load_bias(y_pool)
        wo_sb = wo_pool.tile([128, KC * D], DT, tag="wo", name="wo")
        nc.sync.dma_start(out=wo_sb, in_=wo_ext.ap())
        # proj(b0) half0's A2A completed mid-attn(0); half1's completes
        # ~20us into attn(1) - only queue its matmuls from qh2 on so the PE
        # stream never blocks on an in-flight collective
        attention(1, {
            1: proj_gen(0, 0, wo_sb, bias_sb, g_pool, y_pool),
            2: proj_gen(0, 1, wo_sb, bias_sb, g_pool, y_pool),
        }, rates=(1, 1, 1, 1))
        drain(proj_gen(1, 0, wo_sb, bias_sb, g_pool, y_pool))
        drain(proj_gen(1, 1, wo_sb, bias_sb, g_pool, y_pool))

    nc.finalize()
    return nc


def _prep_in_maps(x, w_qkv, w_out, b_out):
    if USE_BF16:
        import ml_dtypes
        dt = ml_dtypes.bfloat16
    else:
        dt = np.float32
    x = np.asarray(x, dtype=np.float32)
    w_qkv = np.asarray(w_qkv, dtype=np.float32)
    w_out = np.asarray(w_out, dtype=np.float32)
    b_out = np.ascontiguousarray(b_out, dtype=np.float32)

    xT = np.ascontiguousarray(
        np.stack([x[0].T, x[1].T]).reshape(2, KC, 128, S).astype(dt)
    )
    # w_out rows grouped per sender s (heads 2s, 2s+1) = natural row order,
    # packed so the whole thing is ONE [128, 8192] transfer
    wo = np.ascontiguousarray(
        w_out.reshape(KC, 128, D).transpose(1, 0, 2).reshape(128, KC * D).astype(dt)
    )
    in_maps = []
    for c in range(NCORES):
        c0 = c * HPC * DH
        shard = np.concatenate(
            [
                w_qkv[:, c0:c0 + 128],
                w_qkv[:, D + c0:D + c0 + 128],
                w_qkv[:, 2 * D + c0:2 * D + c0 + 128],
            ],
            axis=1,
        )  # [1024, 384]
        wq = np.ascontiguousarray(
            shard.reshape(KC, 128, 3 * 128).transpose(1, 0, 2).reshape(128, -1).astype(dt)
        )
        in_maps.append({"xT": xT, "w_qkv": wq, "w_out": wo, "b_out": b_out})
    return in_maps


def _run(x, w_qkv, w_out, b_out, trace=False):
    _ensure_paths()
    from concourse.bass_utils import run_bass_kernel_spmd

    if "nc" not in _CACHE:
        _CACHE["nc"] = _build_nc()
    nc = _CACHE["nc"]
    in_maps = _prep_in_maps(x, w_qkv, w_out, b_out)
    res = run_bass_kernel_spmd(nc, in_maps, list(range(NCORES)), trace=trace)
    out = np.empty((2, S, D), dtype=np.float32)
    for c in range(NCORES):
        o = np.asarray(res.results[c]["out"], dtype=np.float32)
        for b in range(2):
            for half in range(2):
                t0 = half * 1024 + c * 128
                out[b, t0:t0 + 128, :] = o[b, half]
    return out, res


def kernel(x, w_qkv, w_out, b_out):
    out, _ = _run(x, w_qkv, w_out, b_out, trace=False)
    return out
